# revision 10
# baseline (speedup 1.0000x reference)
"""MoE router gate kernel for Trainium2 (Bass/Tile), 8-core data-parallel,
two-phase (screen + selective rescore) implementation.

Computes, for x[16384, 7168], weight[256, 7168], bias[256]:
    scores  = sigmoid(x @ weight.T)
    biased  = scores + bias
    indices = top8(biased)                        (descending, int32)
    weights = scores[indices] / sum * 2.5         (float32)

Sharding: data-parallel over tokens (2048 tokens/core = 16 tiles of 128),
weight/bias replicated.

Two device programs per call:

  P1 (screen): fp16 main matmul only (xh = fp16(x*16), wh = fp16(w*64); the
  fp16 products accumulate exactly in fp32 PSUM, so score error is the
  representation error ~2^-11.5 in pre-sigmoid units).  Epilogue computes the
  full top-8 weights/indices for every token PLUS an ambiguity measure per
  token: the minimum consecutive gap among the top-9 biased scores (internal
  top-8 order swaps corrupt the index output too, so all eight boundaries
  matter, not just 8-vs-9).  Per 128-token tile the 16 smallest-gap tokens
  are selected on-device (PE transpose of the gap column + two DVE max8
  rounds) and exported as a map, together with the raw fp32 PSUM scores.

  P2 (rescore): host gathers the selected 256 tokens/core worth of fp8 data
  (fp8(xh) and fp8 of the x residual, from the prep arrays; no device gather
  -- register-offset APs crash this runtime) and P2 computes the fp8
  DoubleRow correction fp8(xh)*wl8 + xl8*wh8 for just those tokens against
  all 256 experts ([exp, slot] orientation, weights stationary), adds it to
  the gathered raw scores, and redoes sigmoid/top-8.  Host overwrites the
  rescored rows.  Rescored rows have exactly the old full-k3 accuracy
  (~2^-15), and the numpy simulation of this pipeline reproduces the full-k3
  error (10/16384 mismatched rows, rel err 4.6e-3) at cap=16 per tile.

  PE cost: P1 = 16 tiles * 56 chunks * 256 moving cols = 229376 cyc
  (~118us at the measured ~1.95GHz; slope measures ~122-128us).  P2
  measures ~7.5us: in this orientation (w8 stationary, x8 moving) the
  DoubleRow matmul streams 2 output columns/cycle and its stationary loads
  run at 2 rows/cycle, so 2 halves * 56 chunks * 128 cyc = 14336 cyc --
  half of what the old kernel's moving-rate model assumed for DR.  The old
  single-program kernel (fp16 main + full-token DR corrections, ACT-cast
  feeding the DR stationary) measured ~231us; this two-phase split
  measures ~135us total with identical accuracy (10/16384 mismatched
  rows, rel err 4.74e-3).  DMA drops from 3B to 2B per x element (xl8
  never ships in full; fp8 data only crosses for the 256 slots/core).
"""

import os
from concurrent.futures import ThreadPoolExecutor

import numpy as np

TOKENS = 16384
DIM = 7168
NEXP = 256
TOPK = 8
ROUTE_SCALE = 2.5
NCORES = 8
TPC = TOKENS // NCORES          # tokens per core: 2048
P = 128                         # partitions / tile height
NTILES = TPC // P               # 16 token tiles per core
KC = DIM // P                   # 56 contraction chunks
CAP = 16                        # rescored tokens per tile
NSLOT = NTILES * CAP            # rescored tokens per core: 256

X_SCALE = 16.0   # keep x_lo out of fp16-denormal range
W_SCALE = 64.0   # keep w_lo out of fp16-denormal range
S_XL = 512.0     # scale of fp8(x residual)
S_WH = 8.0       # scale of fp8(w)
S_WL = S_XL * S_WH * 1.0  # scale of fp8(w residual); must equal S_XL*S_WH
SIG_SCALE = 1.0 / (X_SCALE * W_SCALE)

XBUFS = int(os.environ.get("GATE_XBUFS", "3"))
LOOKAHEAD = int(os.environ.get("GATE_LOOKAHEAD", "2"))
PSBUFS = int(os.environ.get("GATE_PSBUFS", "4"))


def _build_p1(reps=1):
    """Screen pass: fp16 scores, per-token top-8 + ambiguity selection."""
    import concourse.bacc as bacc
    import concourse.mybir as mybir
    import concourse.tile as tile

    f32 = mybir.dt.float32
    f16 = mybir.dt.float16
    u32 = mybir.dt.uint32

    nc = bacc.Bacc(
        "TRN2",
        target_bir_lowering=False,
        debug=False,
        enable_asserts=False,
        num_devices=NCORES,
    )

    xh_d = nc.dram_tensor("xh", [NTILES, P, KC, P], f16, kind="ExternalInput").ap()
    wh_d = nc.dram_tensor("wh", [P, KC, NEXP], f16, kind="ExternalInput").ap()
    bb_d = nc.dram_tensor("bb", [P, NEXP], f32, kind="ExternalInput").ap()
    id_d = nc.dram_tensor("ident", [P, P], f32, kind="ExternalInput").ap()
    ow_d = nc.dram_tensor("out_w", [NTILES, P, TOPK], f32, kind="ExternalOutput").ap()
    oi_d = nc.dram_tensor("out_i", [NTILES, P, TOPK], u32, kind="ExternalOutput").ap()
    sr_d = nc.dram_tensor("sraw", [NTILES, P, NEXP], f32, kind="ExternalOutput").ap()
    om_d = nc.dram_tensor("out_map", [NTILES, CAP], u32, kind="ExternalOutput").ap()

    with tile.TileContext(nc) as tc:
        with (
            tc.tile_pool(name="const", bufs=1) as const_pool,
            tc.tile_pool(name="xin", bufs=XBUFS) as x_pool,
            tc.tile_pool(name="psum", bufs=PSBUFS, space="PSUM") as ps_pool,
            tc.tile_pool(name="epi", bufs=3) as ep_pool,
        ):
            wh_sb = const_pool.tile([P, KC, NEXP], f16)
            nc.sync.dma_start(wh_sb[:], wh_d)
            bb_sb = const_pool.tile([P, NEXP], f32)
            nc.sync.dma_start(bb_sb[:], bb_d)
            id_sb = const_pool.tile([P, P], f32)
            nc.sync.dma_start(id_sb[:], id_d)
            # one negated-min-gap column per tile (last rep's values win)
            gapcol = const_pool.tile([P, NTILES], f32)

            seq = [b for _ in range(reps) for b in range(NTILES)]
            loaded = []

            def issue_load(b):
                xh_sb = x_pool.tile([P, KC, P], f16, tag="xh")
                nc.sync.dma_start(xh_sb[:], xh_d[b])
                loaded.append(xh_sb)

            def emit_selection():
                # per-tile top-CAP ambiguous-token selection: transpose the
                # key columns so each tile's 128 keys lie on the free axis of
                # one partition, then two max8 rounds select the 16 largest
                # keys (= smallest gaps) per tile in parallel.
                keyT_ps = ps_pool.tile([NTILES, P], f32, tag="keyT", bufs=1)
                nc.tensor.transpose(keyT_ps[:], gapcol[:], id_sb[:])
                keyT = ep_pool.tile([NTILES, P], f32, tag="keyT_sb")
                nc.vector.tensor_copy(keyT[:], keyT_ps[:])
                map_sb = ep_pool.tile([NTILES, CAP], u32, tag="map")
                k8 = ep_pool.tile([NTILES, 8], f32, tag="k8")
                nc.vector.max(out=k8[:], in_=keyT[:])
                nc.vector.max_index(
                    out=map_sb[:, 0:8], in_max=k8[:], in_values=keyT[:]
                )
                keyT2 = ep_pool.tile([NTILES, P], f32, tag="keyT2")
                nc.vector.match_replace(keyT2[:], k8[:], keyT[:], -1e30)
                k8b = ep_pool.tile([NTILES, 8], f32, tag="k8b")
                nc.vector.max(out=k8b[:], in_=keyT2[:])
                nc.vector.max_index(
                    out=map_sb[:, 8:16], in_max=k8b[:], in_values=keyT2[:]
                )
                nc.sync.dma_start(om_d, map_sb[:])

            for j in range(min(LOOKAHEAD, len(seq))):
                issue_load(seq[j])
            for i, b in enumerate(seq):
                if i + LOOKAHEAD < len(seq):
                    issue_load(seq[i + LOOKAHEAD])
                xh_sb = loaded.pop(0)
                ps = ps_pool.tile([P, NEXP], f32, tag="ps")
                for k in range(KC):
                    nc.tensor.matmul(
                        ps[:],
                        xh_sb[:, k, :],
                        wh_sb[:, k, :],
                        start=(k == 0),
                        stop=(k == KC - 1),
                    )

                # export raw fp32 scores for the host->P2 path
                sraw = ep_pool.tile([P, NEXP], f32, tag="sraw")
                nc.vector.tensor_copy(sraw[:], ps[:])
                nc.sync.dma_start(sr_d[b], sraw[:])

                sig = ep_pool.tile([P, NEXP], f32, tag="sig")
                nc.scalar.activation(
                    sig[:],
                    ps[:],
                    mybir.ActivationFunctionType.Sigmoid,
                    scale=SIG_SCALE,
                )
                biased = ep_pool.tile([P, NEXP], f32, tag="biased")
                nc.vector.tensor_add(biased[:], sig[:], bb_sb[:])

                m9 = ep_pool.tile([P, 9], f32, tag="m9")
                nc.vector.max(out=m9[:, 0:8], in_=biased[:])
                idx = ep_pool.tile([P, TOPK], u32, tag="idx")
                nc.vector.max_index(
                    out=idx[:], in_max=m9[:, 0:8], in_values=biased[:]
                )

                # 9th biased value -> min consecutive gap among top-9
                scr = ep_pool.tile([P, NEXP], f32, tag="scr")
                nc.vector.match_replace(scr[:], m9[:, 0:8], biased[:], -1e30)
                nc.vector.tensor_reduce(
                    m9[:, 8:9], scr[:], axis=mybir.AxisListType.X,
                    op=mybir.AluOpType.max,
                )
                gaps = ep_pool.tile([P, TOPK], f32, tag="gaps")
                nc.vector.tensor_sub(gaps[:], m9[:, 0:8], m9[:, 1:9])
                # negate while reducing: key = -mingap = max(-gaps)
                ngaps = ep_pool.tile([P, TOPK], f32, tag="ngaps")
                nc.vector.tensor_scalar(
                    ngaps[:], gaps[:], -1.0, None, op0=mybir.AluOpType.mult
                )
                nc.vector.tensor_reduce(
                    gapcol[:, b : b + 1], ngaps[:], axis=mybir.AxisListType.X,
                    op=mybir.AluOpType.max,
                )

                # weights: gather sigmoid scores at the selected experts
                sel = ep_pool.tile([P, TOPK], f32, tag="sel")
                scratch = ep_pool.tile([P, NEXP], f32, tag="scratch")
                for j in range(TOPK):
                    nc.vector.scalar_tensor_tensor(
                        out=scratch[:],
                        in0=biased[:],
                        scalar=m9[:, j : j + 1],
                        in1=sig[:],
                        op0=mybir.AluOpType.is_equal,
                        op1=mybir.AluOpType.mult,
                        accum_out=sel[:, j : j + 1],
                    )
                ssum = ep_pool.tile([P, 1], f32, tag="ssum")
                nc.vector.tensor_reduce(
                    ssum[:], sel[:], axis=mybir.AxisListType.X,
                    op=mybir.AluOpType.add,
                )
                rec = ep_pool.tile([P, 1], f32, tag="rec")
                nc.vector.reciprocal(rec[:], ssum[:])
                wout = ep_pool.tile([P, TOPK], f32, tag="wout")
                nc.vector.tensor_scalar(
                    wout[:],
                    sel[:],
                    rec[:],
                    ROUTE_SCALE,
                    op0=mybir.AluOpType.mult,
                    op1=mybir.AluOpType.mult,
                )
                nc.sync.dma_start(ow_d[b], wout[:])
                nc.sync.dma_start(oi_d[b], idx[:])
                if (i + 1) % NTILES == 0:
                    emit_selection()

    nc.compile()
    return nc


def _build_p2(reps=1):
    """Rescore pass: fp8 DoubleRow corrections for NSLOT gathered tokens."""
    import concourse.bacc as bacc
    import concourse.mybir as mybir
    import concourse.tile as tile

    f32 = mybir.dt.float32
    f8 = mybir.dt.float8e4
    u32 = mybir.dt.uint32

    nc = bacc.Bacc(
        "TRN2",
        target_bir_lowering=False,
        debug=False,
        enable_asserts=False,
        num_devices=NCORES,
    )

    # w8[:, 0] = wl8 (pairs fp8(xh)), w8[:, 1] = wh8 (pairs xl8)
    w8_d = nc.dram_tensor("w8", [P, 2, KC, NEXP], f8, kind="ExternalInput").ap()
    xg_d = nc.dram_tensor("x8g", [P, 2, KC, NSLOT], f8, kind="ExternalInput").ap()
    sg_d = nc.dram_tensor("sgT", [P, 2, NSLOT], f32, kind="ExternalInput").ap()
    bc_d = nc.dram_tensor("bias_col", [P, 2], f32, kind="ExternalInput").ap()
    id_d = nc.dram_tensor("ident", [P, P], f32, kind="ExternalInput").ap()
    ow_d = nc.dram_tensor("ow2", [2, P, TOPK], f32, kind="ExternalOutput").ap()
    oi_d = nc.dram_tensor("oi2", [2, P, TOPK], u32, kind="ExternalOutput").ap()

    NST = NSLOT // P  # slot tiles (2)

    with tile.TileContext(nc) as tc:
        with (
            tc.tile_pool(name="const", bufs=1) as const_pool,
            tc.tile_pool(name="psum", bufs=2, space="PSUM") as ps_pool,
            tc.tile_pool(name="epi", bufs=2) as ep_pool,
        ):
            w8_sb = const_pool.tile([P, 2, KC, NEXP], f8)
            nc.sync.dma_start(w8_sb[:], w8_d)
            xg_sb = const_pool.tile([P, 2, KC, NSLOT], f8)
            nc.sync.dma_start(xg_sb[:], xg_d)
            sg_sb = const_pool.tile([P, 2, NSLOT], f32)
            nc.sync.dma_start(sg_sb[:], sg_d)
            bc_sb = const_pool.tile([P, 2], f32)
            nc.sync.dma_start(bc_sb[:], bc_d)
            id_sb = const_pool.tile([P, P], f32)
            nc.sync.dma_start(id_sb[:], id_d)

            for _ in range(reps):
                sigT = [
                    ep_pool.tile([P, NEXP], f32, tag=f"sigT{st}",
                                 name=f"sigT{st}")
                    for st in range(NST)
                ]
                biasedT = [
                    ep_pool.tile([P, NEXP], f32, tag=f"bT{st}",
                                 name=f"bT{st}")
                    for st in range(NST)
                ]
                for h in range(2):
                    psc = ps_pool.tile([P, NSLOT], f32, tag="psc")
                    for k in range(KC):
                        nc.tensor.matmul(
                            psc[:],
                            w8_sb[:, :, k, h * P : (h + 1) * P],
                            xg_sb[:, :, k, :],
                            start=(k == 0),
                            stop=(k == KC - 1),
                            perf_mode=mybir.MatmulPerfMode.DoubleRow,
                        )
                    # s2 = sraw + corr/S_WL ; sig2 = sigmoid(s2/(16*64))
                    corr = ep_pool.tile([P, NSLOT], f32, tag="corr")
                    nc.vector.tensor_scalar(
                        corr[:], psc[:], 1.0 / S_WL, None, op0=mybir.AluOpType.mult
                    )
                    s2 = ep_pool.tile([P, NSLOT], f32, tag="s2")
                    nc.vector.tensor_add(s2[:], corr[:], sg_sb[:, h, :])
                    sig2 = ep_pool.tile([P, NSLOT], f32, tag="sig2")
                    nc.scalar.activation(
                        sig2[:],
                        s2[:],
                        mybir.ActivationFunctionType.Sigmoid,
                        scale=SIG_SCALE,
                    )
                    b2 = ep_pool.tile([P, NSLOT], f32, tag="b2")
                    nc.vector.tensor_scalar(
                        b2[:], sig2[:], bc_sb[:, h : h + 1], None,
                        op0=mybir.AluOpType.add,
                    )
                    # transpose [exp, slot] -> [slot, exp] per slot-tile;
                    # expert half h lands in columns [h*128, (h+1)*128)
                    for st in range(NST):
                        tp = ps_pool.tile([P, P], f32, tag="tp")
                        nc.tensor.transpose(
                            tp[:], sig2[:, st * P : (st + 1) * P], id_sb[:]
                        )
                        nc.vector.tensor_copy(
                            sigT[st][:, h * P : (h + 1) * P], tp[:]
                        )
                        tp2 = ps_pool.tile([P, P], f32, tag="tp2")
                        nc.tensor.transpose(
                            tp2[:], b2[:, st * P : (st + 1) * P], id_sb[:]
                        )
                        nc.vector.tensor_copy(
                            biasedT[st][:, h * P : (h + 1) * P], tp2[:]
                        )

                for st in range(NST):
                    bT = biasedT[st][:]
                    gT = sigT[st][:]
                    max8 = ep_pool.tile([P, TOPK], f32, tag="max8")
                    nc.vector.max(out=max8[:], in_=bT)
                    idx = ep_pool.tile([P, TOPK], u32, tag="idx")
                    nc.vector.max_index(out=idx[:], in_max=max8[:], in_values=bT)
                    sel = ep_pool.tile([P, TOPK], f32, tag="sel")
                    scratch = ep_pool.tile([P, NEXP], f32, tag="scratch")
                    for j in range(TOPK):
                        nc.vector.scalar_tensor_tensor(
                            out=scratch[:],
                            in0=bT,
                            scalar=max8[:, j : j + 1],
                            in1=gT,
                            op0=mybir.AluOpType.is_equal,
                            op1=mybir.AluOpType.mult,
                            accum_out=sel[:, j : j + 1],
                        )
                    ssum = ep_pool.tile([P, 1], f32, tag="ssum")
                    nc.vector.tensor_reduce(
                        ssum[:], sel[:], axis=mybir.AxisListType.X,
                        op=mybir.AluOpType.add,
                    )
                    rec = ep_pool.tile([P, 1], f32, tag="rec")
                    nc.vector.reciprocal(rec[:], ssum[:])
                    wout = ep_pool.tile([P, TOPK], f32, tag="wout")
                    nc.vector.tensor_scalar(
                        wout[:],
                        sel[:],
                        rec[:],
                        ROUTE_SCALE,
                        op0=mybir.AluOpType.mult,
                        op1=mybir.AluOpType.mult,
                    )
                    nc.sync.dma_start(ow_d[st], wout[:])
                    nc.sync.dma_start(oi_d[st], idx[:])

    nc.compile()
    return nc


def _tile_x(x_shard):
    # [2048, D] -> [16, 128(tok), 56(d_out), 128(d_in)] -> [16, 128(d_in), 56, 128(tok)]
    return x_shard.reshape(NTILES, P, KC, P).transpose(0, 3, 2, 1)


_IDENT = np.eye(P, dtype=np.float32)


def _prep_core(x_shard, wh_t, bb):
    """P1 inputs + host-side tiled arrays kept for the P2 gather."""
    import ml_dtypes

    f8 = ml_dtypes.float8_e4m3
    xs = (x_shard * X_SCALE).astype(np.float32)
    xh = xs.astype(np.float16)
    xl = xs - xh.astype(np.float32)
    xh_t = np.ascontiguousarray(_tile_x(xh))
    xl8_t = np.ascontiguousarray(_tile_x((xl * S_XL).astype(f8)))
    p1_in = {"xh": xh_t, "wh": wh_t, "bb": bb, "ident": _IDENT}
    return p1_in, xh_t, xl8_t


def _prep_all(x, w, bias):
    import ml_dtypes

    f8 = ml_dtypes.float8_e4m3

    def _tile_w(warr):
        # [256, 7168] -> [128(d_in), 56(d_out), 256(exp)]
        return np.ascontiguousarray(warr.reshape(NEXP, KC, P).transpose(2, 1, 0))

    ws = (w * W_SCALE).astype(np.float32)
    wh = ws.astype(np.float16)
    wl = ws - wh.astype(np.float32)
    wh_t = _tile_w(wh)
    wl8 = _tile_w((wl * S_WL).astype(f8))             # pairs fp8(xh)
    wh8 = _tile_w((ws * S_WH).astype(f8))             # pairs xl8
    w8 = np.ascontiguousarray(np.stack([wl8, wh8], axis=1))
    bb = np.ascontiguousarray(np.broadcast_to(bias, (P, NEXP)).astype(np.float32))
    # bias_col[p, h] = bias[h*128 + p]
    bias_col = np.ascontiguousarray(bias.reshape(2, P).T.astype(np.float32))

    with ThreadPoolExecutor(NCORES) as pool:
        cores = list(
            pool.map(
                lambda c: _prep_core(x[c * TPC : (c + 1) * TPC], wh_t, bb),
                range(NCORES),
            )
        )
    return cores, w8, bias_col


def _gather_p2_inputs(core_prep, p1_out, w8, bias_col):
    """Host gather of the selected tokens' fp8 data + raw scores."""
    import ml_dtypes

    f8 = ml_dtypes.float8_e4m3
    _, xh_t, xl8_t = core_prep
    m = np.asarray(p1_out["out_map"], np.int64)        # [16, 16] token-in-tile
    tiles = np.repeat(np.arange(NTILES), CAP)          # [256]
    toks = m.reshape(-1)                               # [256]
    # [256, 128, 56] -> [128, 56, 256]
    xh_g = xh_t[tiles, :, :, toks].transpose(1, 2, 0)
    xl8_g = xl8_t[tiles, :, :, toks].transpose(1, 2, 0)
    x8g = np.empty((P, 2, KC, NSLOT), f8)
    x8g[:, 0] = xh_g.astype(f8)
    x8g[:, 1] = xl8_g
    sraw = np.asarray(p1_out["sraw"])                  # [16, 128, 256]
    sg = sraw[tiles, toks]                             # [256 slots, 256 exp]
    sgT = np.ascontiguousarray(
        sg.T.reshape(2, P, NSLOT).transpose(1, 0, 2)
    )  # [128, 2, 256]
    return {
        "w8": w8,
        "x8g": np.ascontiguousarray(x8g),
        "sgT": sgT,
        "bias_col": bias_col,
        "ident": _IDENT,
    }, tiles, toks


def _merge(p1_results, p2_results, maps):
    weights = np.concatenate(
        [np.asarray(r["out_w"]).reshape(TPC, TOPK) for r in p1_results], axis=0
    ).astype(np.float32)
    indices = np.concatenate(
        [np.asarray(r["out_i"]).reshape(TPC, TOPK) for r in p1_results], axis=0
    ).astype(np.int32)
    for c, (r2, (tiles, toks)) in enumerate(zip(p2_results, maps)):
        rows = c * TPC + tiles * P + toks
        weights[rows] = np.asarray(r2["ow2"]).reshape(NSLOT, TOPK)
        indices[rows] = np.asarray(r2["oi2"]).reshape(NSLOT, TOPK).astype(np.int32)
    return weights, indices


def kernel(**inputs):
    from concourse.bass_utils import run_bass_kernel_spmd

    x = np.ascontiguousarray(np.asarray(inputs["x"], dtype=np.float32))
    w = np.ascontiguousarray(np.asarray(inputs["weight"], dtype=np.float32))
    bias = np.asarray(inputs["bias"], dtype=np.float32)

    cores, w8, bias_col = _prep_all(x, w, bias)

    nc1 = _build_p1()
    r1 = run_bass_kernel_spmd(
        nc1, [c[0] for c in cores], core_ids=list(range(NCORES)), trace=False
    ).results

    p2_maps = []
    p2_ins = []
    for c in range(NCORES):
        p2_in, tiles, toks = _gather_p2_inputs(cores[c], r1[c], w8, bias_col)
        p2_ins.append(p2_in)
        p2_maps.append((tiles, toks))

    nc2 = _build_p2()
    r2 = run_bass_kernel_spmd(
        nc2, p2_ins, core_ids=list(range(NCORES)), trace=False
    ).results

    return _merge(r1, r2, p2_maps)


# revision 18
# speedup vs baseline: 1.1571x; 1.1571x over previous
"""MoE router gate kernel for Trainium2 (Bass/Tile), 8-core data-parallel,
two-phase (screen + selective rescore) implementation.

Computes, for x[16384, 7168], weight[256, 7168], bias[256]:
    scores  = sigmoid(x @ weight.T)
    biased  = scores + bias
    indices = top8(biased)                        (descending, int32)
    weights = scores[indices] / sum * 2.5         (float32)

Sharding: data-parallel over tokens (2048 tokens/core = 16 tiles of 128),
weight/bias replicated.

Two device programs per call:

  P1 (screen): fp16 main matmul only (xh = fp16(x*16), wh = fp16(w*64); the
  fp16 products accumulate exactly in fp32 PSUM, so score error is the
  representation error ~2^-11.5 in pre-sigmoid units).  Epilogue computes the
  full top-8 weights/indices for every token PLUS an ambiguity measure per
  token: the minimum consecutive gap among the top-9 biased scores (internal
  top-8 order swaps corrupt the index output too, so all eight boundaries
  matter, not just 8-vs-9).  Per 128-token tile the 16 smallest-gap tokens
  are selected on-device (PE transpose of the gap column + two DVE max8
  rounds) and exported as a map, together with the raw fp32 PSUM scores.

  P2 (rescore): host gathers the selected 256 tokens/core worth of fp8 data
  (fp8(xh) and fp8 of the x residual, from the prep arrays; no device gather
  -- register-offset APs crash this runtime) and P2 computes the fp8
  DoubleRow correction fp8(xh)*wl8 + xl8*wh8 for just those tokens against
  all 256 experts ([exp, slot] orientation, weights stationary), adds it to
  the gathered raw scores, and redoes sigmoid/top-8.  Host overwrites the
  rescored rows.  Rescored rows have exactly the old full-k3 accuracy
  (~2^-15), and the numpy simulation of this pipeline reproduces the full-k3
  error (10/16384 mismatched rows, rel err 4.6e-3) at cap=16 per tile.

  PE cost: P1 = 16 tiles * 56 chunks * 256 moving cols = 229376 cyc
  (~118us at the measured ~1.95GHz; slope measures ~122-128us).  P2
  measures ~7.5us: in this orientation (w8 stationary, x8 moving) the
  DoubleRow matmul streams 2 output columns/cycle and its stationary loads
  run at 2 rows/cycle, so 2 halves * 56 chunks * 128 cyc = 14336 cyc --
  half of what the old kernel's moving-rate model assumed for DR.  The old
  single-program kernel (fp16 main + full-token DR corrections, ACT-cast
  feeding the DR stationary) measured ~231us; this two-phase split
  measures ~135us total with identical accuracy (10/16384 mismatched
  rows, rel err 4.74e-3).  DMA drops from 3B to 2B per x element (xl8
  never ships in full; fp8 data only crosses for the 256 slots/core).
"""

import os
from concurrent.futures import ThreadPoolExecutor

import numpy as np

TOKENS = 16384
DIM = 7168
NEXP = 256
TOPK = 8
ROUTE_SCALE = 2.5
NCORES = 8
TPC = TOKENS // NCORES          # tokens per core: 2048
P = 128                         # partitions / tile height
NTILES = TPC // P               # 16 token tiles per core
KC = DIM // P                   # 56 contraction chunks
CAP = 16                        # rescored tokens per tile
NSLOT = NTILES * CAP            # rescored tokens per core: 256

X_SCALE = 16.0   # keep x_lo out of fp16-denormal range
W_SCALE = 64.0   # keep w_lo out of fp16-denormal range
S_XL = 512.0     # scale of fp8(x residual)
S_WH = 8.0       # scale of fp8(w)
S_WL = S_XL * S_WH * 1.0  # scale of fp8(w residual); must equal S_XL*S_WH
SIG_SCALE = 1.0 / (X_SCALE * W_SCALE)

XBUFS = int(os.environ.get("GATE_XBUFS", "3"))
LOOKAHEAD = int(os.environ.get("GATE_LOOKAHEAD", "2"))
PSBUFS = int(os.environ.get("GATE_PSBUFS", "4"))


def _build_p1(reps=1):
    """Screen pass: fp16 scores, per-token top-8 + ambiguity selection."""
    import concourse.bacc as bacc
    import concourse.mybir as mybir
    import concourse.tile as tile

    f32 = mybir.dt.float32
    f16 = mybir.dt.float16
    u32 = mybir.dt.uint32

    nc = bacc.Bacc(
        "TRN2",
        target_bir_lowering=False,
        debug=False,
        enable_asserts=False,
        num_devices=NCORES,
    )

    xh_d = nc.dram_tensor("xh", [NTILES, P, KC, P], f16, kind="ExternalInput").ap()
    # k-major so the first matmul only waits on one 64KB chunk, not 3.7MB
    wh_d = nc.dram_tensor("wh", [KC, P, NEXP], f16, kind="ExternalInput").ap()
    bb_d = nc.dram_tensor("bb", [P, NEXP], f32, kind="ExternalInput").ap()
    id_d = nc.dram_tensor("ident", [P, P], f32, kind="ExternalInput").ap()
    ow_d = nc.dram_tensor("out_w", [NTILES, P, TOPK], f32, kind="ExternalOutput").ap()
    oi_d = nc.dram_tensor("out_i", [NTILES, P, TOPK], u32, kind="ExternalOutput").ap()
    sr_d = nc.dram_tensor("sraw", [NTILES, P, NEXP], f32, kind="ExternalOutput").ap()
    om_d = nc.dram_tensor("out_map", [NTILES, CAP], u32, kind="ExternalOutput").ap()

    with tile.TileContext(nc) as tc:
        with (
            tc.tile_pool(name="const", bufs=1) as const_pool,
            tc.tile_pool(name="xin", bufs=XBUFS) as x_pool,
            tc.tile_pool(name="psum", bufs=PSBUFS, space="PSUM") as ps_pool,
            tc.tile_pool(name="epi", bufs=3) as ep_pool,
        ):
            wh_sb = const_pool.tile([P, KC, NEXP], f16)
            for k in range(KC):
                nc.sync.dma_start(wh_sb[:, k, :], wh_d[k])
            bb_sb = const_pool.tile([P, NEXP], f32)
            nc.sync.dma_start(bb_sb[:], bb_d)
            id_sb = const_pool.tile([P, P], f32)
            nc.sync.dma_start(id_sb[:], id_d)
            # one negated-min-gap column per tile (last rep's values win)
            gapcol = const_pool.tile([P, NTILES], f32)

            seq = [b for _ in range(reps) for b in range(NTILES)]
            loaded = []

            def issue_load(b):
                xh_sb = x_pool.tile([P, KC, P], f16, tag="xh")
                nc.sync.dma_start(xh_sb[:], xh_d[b])
                loaded.append(xh_sb)

            def emit_selection():
                # per-tile top-CAP ambiguous-token selection: transpose the
                # key columns so each tile's 128 keys lie on the free axis of
                # one partition, then two max8 rounds select the 16 largest
                # keys (= smallest gaps) per tile in parallel.
                keyT_ps = ps_pool.tile([NTILES, P], f32, tag="keyT", bufs=1)
                nc.tensor.transpose(keyT_ps[:], gapcol[:], id_sb[:])
                keyT = ep_pool.tile([NTILES, P], f32, tag="keyT_sb")
                nc.vector.tensor_copy(keyT[:], keyT_ps[:])
                map_sb = ep_pool.tile([NTILES, CAP], u32, tag="map")
                k8 = ep_pool.tile([NTILES, 8], f32, tag="k8")
                nc.vector.max(out=k8[:], in_=keyT[:])
                nc.vector.max_index(
                    out=map_sb[:, 0:8], in_max=k8[:], in_values=keyT[:]
                )
                keyT2 = ep_pool.tile([NTILES, P], f32, tag="keyT2")
                nc.vector.match_replace(keyT2[:], k8[:], keyT[:], -1e30)
                k8b = ep_pool.tile([NTILES, 8], f32, tag="k8b")
                nc.vector.max(out=k8b[:], in_=keyT2[:])
                nc.vector.max_index(
                    out=map_sb[:, 8:16], in_max=k8b[:], in_values=keyT2[:]
                )
                nc.sync.dma_start(om_d, map_sb[:])

            for j in range(min(LOOKAHEAD, len(seq))):
                issue_load(seq[j])
            for i, b in enumerate(seq):
                if i + LOOKAHEAD < len(seq):
                    issue_load(seq[i + LOOKAHEAD])
                xh_sb = loaded.pop(0)
                ps = ps_pool.tile([P, NEXP], f32, tag="ps")
                for k in range(KC):
                    nc.tensor.matmul(
                        ps[:],
                        xh_sb[:, k, :],
                        wh_sb[:, k, :],
                        start=(k == 0),
                        stop=(k == KC - 1),
                    )

                # export raw fp32 scores for the host->P2 path
                sraw = ep_pool.tile([P, NEXP], f32, tag="sraw")
                nc.vector.tensor_copy(sraw[:], ps[:])
                nc.sync.dma_start(sr_d[b], sraw[:])

                sig = ep_pool.tile([P, NEXP], f32, tag="sig")
                nc.scalar.activation(
                    sig[:],
                    ps[:],
                    mybir.ActivationFunctionType.Sigmoid,
                    scale=SIG_SCALE,
                )
                biased = ep_pool.tile([P, NEXP], f32, tag="biased")
                nc.vector.tensor_add(biased[:], sig[:], bb_sb[:])

                m9 = ep_pool.tile([P, 9], f32, tag="m9")
                nc.vector.max(out=m9[:, 0:8], in_=biased[:])
                idx = ep_pool.tile([P, TOPK], u32, tag="idx")
                nc.vector.max_index(
                    out=idx[:], in_max=m9[:, 0:8], in_values=biased[:]
                )

                # 9th biased value -> min consecutive gap among top-9
                scr = ep_pool.tile([P, NEXP], f32, tag="scr")
                nc.vector.match_replace(scr[:], m9[:, 0:8], biased[:], -1e30)
                nc.vector.tensor_reduce(
                    m9[:, 8:9], scr[:], axis=mybir.AxisListType.X,
                    op=mybir.AluOpType.max,
                )
                gaps = ep_pool.tile([P, TOPK], f32, tag="gaps")
                nc.vector.tensor_sub(gaps[:], m9[:, 0:8], m9[:, 1:9])
                # negate while reducing: key = -mingap = max(-gaps)
                ngaps = ep_pool.tile([P, TOPK], f32, tag="ngaps")
                nc.vector.tensor_scalar(
                    ngaps[:], gaps[:], -1.0, None, op0=mybir.AluOpType.mult
                )
                nc.vector.tensor_reduce(
                    gapcol[:, b : b + 1], ngaps[:], axis=mybir.AxisListType.X,
                    op=mybir.AluOpType.max,
                )

                # weights: gather sigmoid scores at the selected experts
                sel = ep_pool.tile([P, TOPK], f32, tag="sel")
                scratch = ep_pool.tile([P, NEXP], f32, tag="scratch")
                for j in range(TOPK):
                    nc.vector.scalar_tensor_tensor(
                        out=scratch[:],
                        in0=biased[:],
                        scalar=m9[:, j : j + 1],
                        in1=sig[:],
                        op0=mybir.AluOpType.is_equal,
                        op1=mybir.AluOpType.mult,
                        accum_out=sel[:, j : j + 1],
                    )
                ssum = ep_pool.tile([P, 1], f32, tag="ssum")
                nc.vector.tensor_reduce(
                    ssum[:], sel[:], axis=mybir.AxisListType.X,
                    op=mybir.AluOpType.add,
                )
                rec = ep_pool.tile([P, 1], f32, tag="rec")
                nc.vector.reciprocal(rec[:], ssum[:])
                wout = ep_pool.tile([P, TOPK], f32, tag="wout")
                nc.vector.tensor_scalar(
                    wout[:],
                    sel[:],
                    rec[:],
                    ROUTE_SCALE,
                    op0=mybir.AluOpType.mult,
                    op1=mybir.AluOpType.mult,
                )
                nc.sync.dma_start(ow_d[b], wout[:])
                nc.sync.dma_start(oi_d[b], idx[:])
                if (i + 1) % NTILES == 0:
                    emit_selection()

    nc.compile()
    return nc


def _build_p2(reps=1):
    """Rescore pass: fp8 DoubleRow corrections for NSLOT gathered tokens."""
    import concourse.bacc as bacc
    import concourse.mybir as mybir
    import concourse.tile as tile

    f32 = mybir.dt.float32
    f8 = mybir.dt.float8e4
    u32 = mybir.dt.uint32

    nc = bacc.Bacc(
        "TRN2",
        target_bir_lowering=False,
        debug=False,
        enable_asserts=False,
        num_devices=NCORES,
    )

    # w8[..., 0, :] = wl8 (pairs fp8(xh)), w8[..., 1, :] = wh8 (pairs xl8);
    # k-major so the first DR matmul only waits on one chunk of each
    w8_d = nc.dram_tensor("w8", [KC, P, 2, NEXP], f8, kind="ExternalInput").ap()
    xg_d = nc.dram_tensor("x8g", [KC, P, 2, NSLOT], f8, kind="ExternalInput").ap()
    sg_d = nc.dram_tensor("sgT", [P, 2, NSLOT], f32, kind="ExternalInput").ap()
    bc_d = nc.dram_tensor("bias_col", [P, 2], f32, kind="ExternalInput").ap()
    id_d = nc.dram_tensor("ident", [P, P], f32, kind="ExternalInput").ap()
    ow_d = nc.dram_tensor("ow2", [2, P, TOPK], f32, kind="ExternalOutput").ap()
    oi_d = nc.dram_tensor("oi2", [2, P, TOPK], u32, kind="ExternalOutput").ap()

    NST = NSLOT // P  # slot tiles (2)

    with tile.TileContext(nc) as tc:
        with (
            tc.tile_pool(name="const", bufs=1) as const_pool,
            tc.tile_pool(name="psum", bufs=2, space="PSUM") as ps_pool,
            tc.tile_pool(name="epi", bufs=2) as ep_pool,
        ):
            w8_sb = const_pool.tile([P, 2, KC, NEXP], f8)
            xg_sb = const_pool.tile([P, 2, KC, NSLOT], f8)
            for k in range(KC):
                nc.sync.dma_start(w8_sb[:, :, k, :], w8_d[k])
                nc.sync.dma_start(xg_sb[:, :, k, :], xg_d[k])
            sg_sb = const_pool.tile([P, 2, NSLOT], f32)
            nc.sync.dma_start(sg_sb[:], sg_d)
            bc_sb = const_pool.tile([P, 2], f32)
            nc.sync.dma_start(bc_sb[:], bc_d)
            id_sb = const_pool.tile([P, P], f32)
            nc.sync.dma_start(id_sb[:], id_d)

            for _ in range(reps):
                sigT = [
                    ep_pool.tile([P, NEXP], f32, tag=f"sigT{st}",
                                 name=f"sigT{st}")
                    for st in range(NST)
                ]
                biasedT = [
                    ep_pool.tile([P, NEXP], f32, tag=f"bT{st}",
                                 name=f"bT{st}")
                    for st in range(NST)
                ]
                for h in range(2):
                    psc = ps_pool.tile([P, NSLOT], f32, tag="psc")
                    for k in range(KC):
                        nc.tensor.matmul(
                            psc[:],
                            w8_sb[:, :, k, h * P : (h + 1) * P],
                            xg_sb[:, :, k, :],
                            start=(k == 0),
                            stop=(k == KC - 1),
                            perf_mode=mybir.MatmulPerfMode.DoubleRow,
                        )
                    # s2 = sraw + corr/S_WL ; sig2 = sigmoid(s2/(16*64))
                    corr = ep_pool.tile([P, NSLOT], f32, tag="corr")
                    nc.vector.tensor_scalar(
                        corr[:], psc[:], 1.0 / S_WL, None, op0=mybir.AluOpType.mult
                    )
                    s2 = ep_pool.tile([P, NSLOT], f32, tag="s2")
                    nc.vector.tensor_add(s2[:], corr[:], sg_sb[:, h, :])
                    sig2 = ep_pool.tile([P, NSLOT], f32, tag="sig2")
                    nc.scalar.activation(
                        sig2[:],
                        s2[:],
                        mybir.ActivationFunctionType.Sigmoid,
                        scale=SIG_SCALE,
                    )
                    b2 = ep_pool.tile([P, NSLOT], f32, tag="b2")
                    nc.vector.tensor_scalar(
                        b2[:], sig2[:], bc_sb[:, h : h + 1], None,
                        op0=mybir.AluOpType.add,
                    )
                    # transpose [exp, slot] -> [slot, exp] per slot-tile;
                    # expert half h lands in columns [h*128, (h+1)*128)
                    for st in range(NST):
                        tp = ps_pool.tile([P, P], f32, tag="tp")
                        nc.tensor.transpose(
                            tp[:], sig2[:, st * P : (st + 1) * P], id_sb[:]
                        )
                        nc.vector.tensor_copy(
                            sigT[st][:, h * P : (h + 1) * P], tp[:]
                        )
                        tp2 = ps_pool.tile([P, P], f32, tag="tp2")
                        nc.tensor.transpose(
                            tp2[:], b2[:, st * P : (st + 1) * P], id_sb[:]
                        )
                        nc.vector.tensor_copy(
                            biasedT[st][:, h * P : (h + 1) * P], tp2[:]
                        )

                for st in range(NST):
                    bT = biasedT[st][:]
                    gT = sigT[st][:]
                    max8 = ep_pool.tile([P, TOPK], f32, tag="max8")
                    nc.vector.max(out=max8[:], in_=bT)
                    idx = ep_pool.tile([P, TOPK], u32, tag="idx")
                    nc.vector.max_index(out=idx[:], in_max=max8[:], in_values=bT)
                    sel = ep_pool.tile([P, TOPK], f32, tag="sel")
                    scratch = ep_pool.tile([P, NEXP], f32, tag="scratch")
                    for j in range(TOPK):
                        nc.vector.scalar_tensor_tensor(
                            out=scratch[:],
                            in0=bT,
                            scalar=max8[:, j : j + 1],
                            in1=gT,
                            op0=mybir.AluOpType.is_equal,
                            op1=mybir.AluOpType.mult,
                            accum_out=sel[:, j : j + 1],
                        )
                    ssum = ep_pool.tile([P, 1], f32, tag="ssum")
                    nc.vector.tensor_reduce(
                        ssum[:], sel[:], axis=mybir.AxisListType.X,
                        op=mybir.AluOpType.add,
                    )
                    rec = ep_pool.tile([P, 1], f32, tag="rec")
                    nc.vector.reciprocal(rec[:], ssum[:])
                    wout = ep_pool.tile([P, TOPK], f32, tag="wout")
                    nc.vector.tensor_scalar(
                        wout[:],
                        sel[:],
                        rec[:],
                        ROUTE_SCALE,
                        op0=mybir.AluOpType.mult,
                        op1=mybir.AluOpType.mult,
                    )
                    nc.sync.dma_start(ow_d[st], wout[:])
                    nc.sync.dma_start(oi_d[st], idx[:])

    nc.compile()
    return nc


def _tile_x(x_shard):
    # [2048, D] -> [16, 128(tok), 56(d_out), 128(d_in)] -> [16, 128(d_in), 56, 128(tok)]
    return x_shard.reshape(NTILES, P, KC, P).transpose(0, 3, 2, 1)


_IDENT = np.eye(P, dtype=np.float32)


def _prep_core(x_shard, wh_t, bb):
    """P1 inputs + host-side tiled arrays kept for the P2 gather."""
    import ml_dtypes

    f8 = ml_dtypes.float8_e4m3
    xs = (x_shard * X_SCALE).astype(np.float32)
    xh = xs.astype(np.float16)
    xl = xs - xh.astype(np.float32)
    xh_t = np.ascontiguousarray(_tile_x(xh))
    xl8_t = np.ascontiguousarray(_tile_x((xl * S_XL).astype(f8)))
    p1_in = {"xh": xh_t, "wh": wh_t, "bb": bb, "ident": _IDENT}
    return p1_in, xh_t, xl8_t


def _k_major(a):
    # [P, (2,) KC, N] -> [KC, P, (2,) N]
    if a.ndim == 3:
        return np.ascontiguousarray(a.transpose(1, 0, 2))
    return np.ascontiguousarray(a.transpose(2, 0, 1, 3))


def _prep_all(x, w, bias):
    import ml_dtypes

    f8 = ml_dtypes.float8_e4m3

    def _tile_w(warr):
        # [256, 7168] -> [128(d_in), 56(d_out), 256(exp)]
        return np.ascontiguousarray(warr.reshape(NEXP, KC, P).transpose(2, 1, 0))

    ws = (w * W_SCALE).astype(np.float32)
    wh = ws.astype(np.float16)
    wl = ws - wh.astype(np.float32)
    wh_t = _k_major(_tile_w(wh))
    wl8 = _tile_w((wl * S_WL).astype(f8))             # pairs fp8(xh)
    wh8 = _tile_w((ws * S_WH).astype(f8))             # pairs xl8
    w8 = _k_major(np.stack([wl8, wh8], axis=1))
    bb = np.ascontiguousarray(np.broadcast_to(bias, (P, NEXP)).astype(np.float32))
    # bias_col[p, h] = bias[h*128 + p]
    bias_col = np.ascontiguousarray(bias.reshape(2, P).T.astype(np.float32))

    with ThreadPoolExecutor(NCORES) as pool:
        cores = list(
            pool.map(
                lambda c: _prep_core(x[c * TPC : (c + 1) * TPC], wh_t, bb),
                range(NCORES),
            )
        )
    return cores, w8, bias_col


def _gather_p2_inputs(core_prep, p1_out, w8, bias_col):
    """Host gather of the selected tokens' fp8 data + raw scores."""
    import ml_dtypes

    f8 = ml_dtypes.float8_e4m3
    _, xh_t, xl8_t = core_prep
    m = np.asarray(p1_out["out_map"], np.int64)        # [16, 16] token-in-tile
    tiles = np.repeat(np.arange(NTILES), CAP)          # [256]
    toks = m.reshape(-1)                               # [256]
    # [256, 128, 56] -> [128, 56, 256]
    xh_g = xh_t[tiles, :, :, toks].transpose(1, 2, 0)
    xl8_g = xl8_t[tiles, :, :, toks].transpose(1, 2, 0)
    x8g = np.empty((P, 2, KC, NSLOT), f8)
    x8g[:, 0] = xh_g.astype(f8)
    x8g[:, 1] = xl8_g
    x8g = _k_major(x8g)
    sraw = np.asarray(p1_out["sraw"])                  # [16, 128, 256]
    sg = sraw[tiles, toks]                             # [256 slots, 256 exp]
    sgT = np.ascontiguousarray(
        sg.T.reshape(2, P, NSLOT).transpose(1, 0, 2)
    )  # [128, 2, 256]
    return {
        "w8": w8,
        "x8g": x8g,
        "sgT": sgT,
        "bias_col": bias_col,
        "ident": _IDENT,
    }, tiles, toks


def _merge(p1_results, p2_results, maps):
    weights = np.concatenate(
        [np.asarray(r["out_w"]).reshape(TPC, TOPK) for r in p1_results], axis=0
    ).astype(np.float32)
    indices = np.concatenate(
        [np.asarray(r["out_i"]).reshape(TPC, TOPK) for r in p1_results], axis=0
    ).astype(np.int32)
    for c, (r2, (tiles, toks)) in enumerate(zip(p2_results, maps)):
        rows = c * TPC + tiles * P + toks
        weights[rows] = np.asarray(r2["ow2"]).reshape(NSLOT, TOPK)
        indices[rows] = np.asarray(r2["oi2"]).reshape(NSLOT, TOPK).astype(np.int32)
    return weights, indices


def kernel(**inputs):
    from concourse.bass_utils import run_bass_kernel_spmd

    x = np.ascontiguousarray(np.asarray(inputs["x"], dtype=np.float32))
    w = np.ascontiguousarray(np.asarray(inputs["weight"], dtype=np.float32))
    bias = np.asarray(inputs["bias"], dtype=np.float32)

    cores, w8, bias_col = _prep_all(x, w, bias)

    nc1 = _build_p1()
    r1 = run_bass_kernel_spmd(
        nc1, [c[0] for c in cores], core_ids=list(range(NCORES)), trace=False
    ).results

    p2_maps = []
    p2_ins = []
    for c in range(NCORES):
        p2_in, tiles, toks = _gather_p2_inputs(cores[c], r1[c], w8, bias_col)
        p2_ins.append(p2_in)
        p2_maps.append((tiles, toks))

    nc2 = _build_p2()
    r2 = run_bass_kernel_spmd(
        nc2, p2_ins, core_ids=list(range(NCORES)), trace=False
    ).results

    return _merge(r1, r2, p2_maps)


# revision 19
# speedup vs baseline: 1.5335x; 1.3253x over previous
"""MoE router gate kernel for Trainium2 (Bass/Tile), 8-core data-parallel,
two-phase (screen + selective rescore) implementation.

Computes, for x[16384, 7168], weight[256, 7168], bias[256]:
    scores  = sigmoid(x @ weight.T)
    biased  = scores + bias
    indices = top8(biased)                        (descending, int32)
    weights = scores[indices] / sum * 2.5         (float32)

Sharding: data-parallel over tokens (2048 tokens/core = 16 tiles of 128),
weight/bias replicated.

Two device programs per call:

  P1 (screen): fp16 main matmul only (xh = fp16(x*16), wh = fp16(w*64); the
  fp16 products accumulate exactly in fp32 PSUM, so score error is the
  representation error ~2^-11.5 in pre-sigmoid units).  Epilogue computes the
  full top-8 weights/indices for every token PLUS an ambiguity measure per
  token: the minimum consecutive gap among the top-9 biased scores (internal
  top-8 order swaps corrupt the index output too, so all eight boundaries
  matter, not just 8-vs-9).  Per 128-token tile the 16 smallest-gap tokens
  are selected on-device (PE transpose of the gap column + two DVE max8
  rounds) and exported as a map, together with the raw fp32 PSUM scores.

  P2 (rescore): host gathers the selected 256 tokens/core worth of fp8 data
  (fp8(xh) and fp8 of the x residual, from the prep arrays; no device gather
  -- register-offset APs crash this runtime) and P2 computes the fp8
  DoubleRow correction fp8(xh)*wl8 + xl8*wh8 for just those tokens against
  all 256 experts ([exp, slot] orientation, weights stationary), adds it to
  the gathered raw scores, and redoes sigmoid/top-8.  Host overwrites the
  rescored rows.  Rescored rows have exactly the old full-k3 accuracy
  (~2^-15), and the numpy simulation of this pipeline reproduces the full-k3
  error (10/16384 mismatched rows, rel err 4.6e-3) at cap=16 per tile.

  PE cost: P1 = 16 tiles * 56 chunks * 256 moving cols = 229376 cyc
  (~118us at the measured ~1.95GHz; slope measures ~122-128us).  P2
  measures ~7.5us: in this orientation (w8 stationary, x8 moving) the
  DoubleRow matmul streams 2 output columns/cycle and its stationary loads
  run at 2 rows/cycle, so 2 halves * 56 chunks * 128 cyc = 14336 cyc --
  half of what the old kernel's moving-rate model assumed for DR.  The old
  single-program kernel (fp16 main + full-token DR corrections, ACT-cast
  feeding the DR stationary) measured ~231us; this two-phase split
  measures ~135us total with identical accuracy (10/16384 mismatched
  rows, rel err 4.74e-3).  DMA drops from 3B to 2B per x element (xl8
  never ships in full; fp8 data only crosses for the 256 slots/core).
"""

import os
from concurrent.futures import ThreadPoolExecutor

import numpy as np

TOKENS = 16384
DIM = 7168
NEXP = 256
TOPK = 8
ROUTE_SCALE = 2.5
NCORES = 8
TPC = TOKENS // NCORES          # tokens per core: 2048
P = 128                         # partitions / tile height
NTILES = TPC // P               # 16 token tiles per core
KC = DIM // P                   # 56 contraction chunks
CAP = 16                        # rescored tokens per tile
NSLOT = NTILES * CAP            # rescored tokens per core: 256

X_SCALE = 16.0   # keep x_lo out of fp16-denormal range
W_SCALE = 64.0   # keep w_lo out of fp16-denormal range
S_XL = 512.0     # scale of fp8(x residual)
S_WH = 8.0       # scale of fp8(w)
S_WL = S_XL * S_WH * 1.0  # scale of fp8(w residual); must equal S_XL*S_WH
SIG_SCALE = 1.0 / (X_SCALE * W_SCALE)

XBUFS = int(os.environ.get("GATE_XBUFS", "4"))
LOOKAHEAD = int(os.environ.get("GATE_LOOKAHEAD", "2"))
PSBUFS = int(os.environ.get("GATE_PSBUFS", "4"))


def _build_p1(reps=1):
    """Screen pass: fp16 scores, per-token top-8 + ambiguity selection."""
    import concourse.bacc as bacc
    import concourse.mybir as mybir
    import concourse.tile as tile

    f32 = mybir.dt.float32
    f16 = mybir.dt.float16
    u32 = mybir.dt.uint32

    nc = bacc.Bacc(
        "TRN2",
        target_bir_lowering=False,
        debug=False,
        enable_asserts=False,
        num_devices=NCORES,
    )

    xh_d = nc.dram_tensor("xh", [NTILES, P, KC, P], f16, kind="ExternalInput").ap()
    # k-major so the first matmul only waits on one 64KB chunk, not 3.7MB
    wh_d = nc.dram_tensor("wh", [KC, P, NEXP], f16, kind="ExternalInput").ap()
    bb_d = nc.dram_tensor("bb", [P, NEXP], f32, kind="ExternalInput").ap()
    id_d = nc.dram_tensor("ident", [P, P], f32, kind="ExternalInput").ap()
    ow_d = nc.dram_tensor("out_w", [NTILES, P, TOPK], f32, kind="ExternalOutput").ap()
    oi_d = nc.dram_tensor("out_i", [NTILES, P, TOPK], u32, kind="ExternalOutput").ap()
    sr_d = nc.dram_tensor("sraw", [NTILES, P, NEXP], f32, kind="ExternalOutput").ap()
    om_d = nc.dram_tensor("out_map", [NTILES, CAP], u32, kind="ExternalOutput").ap()

    with tile.TileContext(nc) as tc:
        with (
            tc.tile_pool(name="const", bufs=1) as const_pool,
            tc.tile_pool(name="xin", bufs=XBUFS) as x_pool,
            tc.tile_pool(name="psum", bufs=PSBUFS, space="PSUM") as ps_pool,
            tc.tile_pool(name="epi", bufs=3) as ep_pool,
        ):
            wh_sb = const_pool.tile([P, KC, NEXP], f16)
            for k in range(KC):
                nc.scalar.dma_start(wh_sb[:, k, :], wh_d[k])
            bb_sb = const_pool.tile([P, NEXP], f32)
            nc.sync.dma_start(bb_sb[:], bb_d)
            id_sb = const_pool.tile([P, P], f32)
            nc.sync.dma_start(id_sb[:], id_d)
            # one negated-min-gap column per tile (last rep's values win)
            gapcol = const_pool.tile([P, NTILES], f32)

            seq = [b for _ in range(reps) for b in range(NTILES)]
            loaded = []

            def issue_load(b):
                xh_sb = x_pool.tile([P, KC, P], f16, tag="xh")
                nc.sync.dma_start(xh_sb[:], xh_d[b])
                loaded.append(xh_sb)

            def emit_selection():
                # per-tile top-CAP ambiguous-token selection: transpose the
                # key columns so each tile's 128 keys lie on the free axis of
                # one partition, then two max8 rounds select the 16 largest
                # keys (= smallest gaps) per tile in parallel.
                keyT_ps = ps_pool.tile([NTILES, P], f32, tag="keyT", bufs=1)
                nc.tensor.transpose(keyT_ps[:], gapcol[:], id_sb[:])
                keyT = ep_pool.tile([NTILES, P], f32, tag="keyT_sb")
                nc.vector.tensor_copy(keyT[:], keyT_ps[:])
                map_sb = ep_pool.tile([NTILES, CAP], u32, tag="map")
                k8 = ep_pool.tile([NTILES, 8], f32, tag="k8")
                nc.vector.max(out=k8[:], in_=keyT[:])
                nc.vector.max_index(
                    out=map_sb[:, 0:8], in_max=k8[:], in_values=keyT[:]
                )
                keyT2 = ep_pool.tile([NTILES, P], f32, tag="keyT2")
                nc.vector.match_replace(keyT2[:], k8[:], keyT[:], -1e30)
                k8b = ep_pool.tile([NTILES, 8], f32, tag="k8b")
                nc.vector.max(out=k8b[:], in_=keyT2[:])
                nc.vector.max_index(
                    out=map_sb[:, 8:16], in_max=k8b[:], in_values=keyT2[:]
                )
                nc.scalar.dma_start(om_d, map_sb[:])

            for j in range(min(LOOKAHEAD, len(seq))):
                issue_load(seq[j])
            for i, b in enumerate(seq):
                if i + LOOKAHEAD < len(seq):
                    issue_load(seq[i + LOOKAHEAD])
                xh_sb = loaded.pop(0)
                ps = ps_pool.tile([P, NEXP], f32, tag="ps")
                for k in range(KC):
                    nc.tensor.matmul(
                        ps[:],
                        xh_sb[:, k, :],
                        wh_sb[:, k, :],
                        start=(k == 0),
                        stop=(k == KC - 1),
                    )

                # export raw fp32 scores for the host->P2 path
                sraw = ep_pool.tile([P, NEXP], f32, tag="sraw")
                nc.vector.tensor_copy(sraw[:], ps[:])
                nc.scalar.dma_start(sr_d[b], sraw[:])

                sig = ep_pool.tile([P, NEXP], f32, tag="sig")
                nc.scalar.activation(
                    sig[:],
                    ps[:],
                    mybir.ActivationFunctionType.Sigmoid,
                    scale=SIG_SCALE,
                )
                biased = ep_pool.tile([P, NEXP], f32, tag="biased")
                nc.vector.tensor_add(biased[:], sig[:], bb_sb[:])

                m9 = ep_pool.tile([P, 9], f32, tag="m9")
                nc.vector.max(out=m9[:, 0:8], in_=biased[:])
                idx = ep_pool.tile([P, TOPK], u32, tag="idx")
                nc.vector.max_index(
                    out=idx[:], in_max=m9[:, 0:8], in_values=biased[:]
                )

                # 9th biased value -> min consecutive gap among top-9
                scr = ep_pool.tile([P, NEXP], f32, tag="scr")
                nc.vector.match_replace(scr[:], m9[:, 0:8], biased[:], -1e30)
                nc.vector.tensor_reduce(
                    m9[:, 8:9], scr[:], axis=mybir.AxisListType.X,
                    op=mybir.AluOpType.max,
                )
                gaps = ep_pool.tile([P, TOPK], f32, tag="gaps")
                nc.vector.tensor_sub(gaps[:], m9[:, 0:8], m9[:, 1:9])
                # negate while reducing: key = -mingap = max(-gaps)
                ngaps = ep_pool.tile([P, TOPK], f32, tag="ngaps")
                nc.vector.tensor_scalar(
                    ngaps[:], gaps[:], -1.0, None, op0=mybir.AluOpType.mult
                )
                nc.vector.tensor_reduce(
                    gapcol[:, b : b + 1], ngaps[:], axis=mybir.AxisListType.X,
                    op=mybir.AluOpType.max,
                )

                # weights: gather sigmoid scores at the selected experts
                sel = ep_pool.tile([P, TOPK], f32, tag="sel")
                scratch = ep_pool.tile([P, NEXP], f32, tag="scratch")
                for j in range(TOPK):
                    nc.vector.scalar_tensor_tensor(
                        out=scratch[:],
                        in0=biased[:],
                        scalar=m9[:, j : j + 1],
                        in1=sig[:],
                        op0=mybir.AluOpType.is_equal,
                        op1=mybir.AluOpType.mult,
                        accum_out=sel[:, j : j + 1],
                    )
                ssum = ep_pool.tile([P, 1], f32, tag="ssum")
                nc.vector.tensor_reduce(
                    ssum[:], sel[:], axis=mybir.AxisListType.X,
                    op=mybir.AluOpType.add,
                )
                rec = ep_pool.tile([P, 1], f32, tag="rec")
                nc.vector.reciprocal(rec[:], ssum[:])
                wout = ep_pool.tile([P, TOPK], f32, tag="wout")
                nc.vector.tensor_scalar(
                    wout[:],
                    sel[:],
                    rec[:],
                    ROUTE_SCALE,
                    op0=mybir.AluOpType.mult,
                    op1=mybir.AluOpType.mult,
                )
                nc.scalar.dma_start(ow_d[b], wout[:])
                nc.scalar.dma_start(oi_d[b], idx[:])
                if (i + 1) % NTILES == 0:
                    emit_selection()

    nc.compile()
    return nc


def _build_p2(reps=1):
    """Rescore pass: fp8 DoubleRow corrections for NSLOT gathered tokens."""
    import concourse.bacc as bacc
    import concourse.mybir as mybir
    import concourse.tile as tile

    f32 = mybir.dt.float32
    f8 = mybir.dt.float8e4
    u32 = mybir.dt.uint32

    nc = bacc.Bacc(
        "TRN2",
        target_bir_lowering=False,
        debug=False,
        enable_asserts=False,
        num_devices=NCORES,
    )

    # w8[..., 0, :] = wl8 (pairs fp8(xh)), w8[..., 1, :] = wh8 (pairs xl8);
    # k-major so the first DR matmul only waits on one chunk of each
    w8_d = nc.dram_tensor("w8", [KC, P, 2, NEXP], f8, kind="ExternalInput").ap()
    xg_d = nc.dram_tensor("x8g", [KC, P, 2, NSLOT], f8, kind="ExternalInput").ap()
    sg_d = nc.dram_tensor("sgT", [P, 2, NSLOT], f32, kind="ExternalInput").ap()
    bc_d = nc.dram_tensor("bias_col", [P, 2], f32, kind="ExternalInput").ap()
    id_d = nc.dram_tensor("ident", [P, P], f32, kind="ExternalInput").ap()
    ow_d = nc.dram_tensor("ow2", [2, P, TOPK], f32, kind="ExternalOutput").ap()
    oi_d = nc.dram_tensor("oi2", [2, P, TOPK], u32, kind="ExternalOutput").ap()

    NST = NSLOT // P  # slot tiles (2)

    with tile.TileContext(nc) as tc:
        with (
            tc.tile_pool(name="const", bufs=1) as const_pool,
            tc.tile_pool(name="psum", bufs=2, space="PSUM") as ps_pool,
            tc.tile_pool(name="epi", bufs=2) as ep_pool,
        ):
            w8_sb = const_pool.tile([P, 2, KC, NEXP], f8)
            xg_sb = const_pool.tile([P, 2, KC, NSLOT], f8)
            for k in range(KC):
                nc.sync.dma_start(w8_sb[:, :, k, :], w8_d[k])
                nc.sync.dma_start(xg_sb[:, :, k, :], xg_d[k])
            sg_sb = const_pool.tile([P, 2, NSLOT], f32)
            nc.sync.dma_start(sg_sb[:], sg_d)
            bc_sb = const_pool.tile([P, 2], f32)
            nc.sync.dma_start(bc_sb[:], bc_d)
            id_sb = const_pool.tile([P, P], f32)
            nc.sync.dma_start(id_sb[:], id_d)

            for _ in range(reps):
                sigT = [
                    ep_pool.tile([P, NEXP], f32, tag=f"sigT{st}",
                                 name=f"sigT{st}")
                    for st in range(NST)
                ]
                biasedT = [
                    ep_pool.tile([P, NEXP], f32, tag=f"bT{st}",
                                 name=f"bT{st}")
                    for st in range(NST)
                ]
                for h in range(2):
                    psc = ps_pool.tile([P, NSLOT], f32, tag="psc")
                    for k in range(KC):
                        nc.tensor.matmul(
                            psc[:],
                            w8_sb[:, :, k, h * P : (h + 1) * P],
                            xg_sb[:, :, k, :],
                            start=(k == 0),
                            stop=(k == KC - 1),
                            perf_mode=mybir.MatmulPerfMode.DoubleRow,
                        )
                    # s2 = sraw + corr/S_WL ; sig2 = sigmoid(s2/(16*64))
                    corr = ep_pool.tile([P, NSLOT], f32, tag="corr")
                    nc.vector.tensor_scalar(
                        corr[:], psc[:], 1.0 / S_WL, None, op0=mybir.AluOpType.mult
                    )
                    s2 = ep_pool.tile([P, NSLOT], f32, tag="s2")
                    nc.vector.tensor_add(s2[:], corr[:], sg_sb[:, h, :])
                    sig2 = ep_pool.tile([P, NSLOT], f32, tag="sig2")
                    nc.scalar.activation(
                        sig2[:],
                        s2[:],
                        mybir.ActivationFunctionType.Sigmoid,
                        scale=SIG_SCALE,
                    )
                    b2 = ep_pool.tile([P, NSLOT], f32, tag="b2")
                    nc.vector.tensor_scalar(
                        b2[:], sig2[:], bc_sb[:, h : h + 1], None,
                        op0=mybir.AluOpType.add,
                    )
                    # transpose [exp, slot] -> [slot, exp] per slot-tile;
                    # expert half h lands in columns [h*128, (h+1)*128)
                    for st in range(NST):
                        tp = ps_pool.tile([P, P], f32, tag="tp")
                        nc.tensor.transpose(
                            tp[:], sig2[:, st * P : (st + 1) * P], id_sb[:]
                        )
                        nc.vector.tensor_copy(
                            sigT[st][:, h * P : (h + 1) * P], tp[:]
                        )
                        tp2 = ps_pool.tile([P, P], f32, tag="tp2")
                        nc.tensor.transpose(
                            tp2[:], b2[:, st * P : (st + 1) * P], id_sb[:]
                        )
                        nc.vector.tensor_copy(
                            biasedT[st][:, h * P : (h + 1) * P], tp2[:]
                        )

                for st in range(NST):
                    bT = biasedT[st][:]
                    gT = sigT[st][:]
                    max8 = ep_pool.tile([P, TOPK], f32, tag="max8")
                    nc.vector.max(out=max8[:], in_=bT)
                    idx = ep_pool.tile([P, TOPK], u32, tag="idx")
                    nc.vector.max_index(out=idx[:], in_max=max8[:], in_values=bT)
                    sel = ep_pool.tile([P, TOPK], f32, tag="sel")
                    scratch = ep_pool.tile([P, NEXP], f32, tag="scratch")
                    for j in range(TOPK):
                        nc.vector.scalar_tensor_tensor(
                            out=scratch[:],
                            in0=bT,
                            scalar=max8[:, j : j + 1],
                            in1=gT,
                            op0=mybir.AluOpType.is_equal,
                            op1=mybir.AluOpType.mult,
                            accum_out=sel[:, j : j + 1],
                        )
                    ssum = ep_pool.tile([P, 1], f32, tag="ssum")
                    nc.vector.tensor_reduce(
                        ssum[:], sel[:], axis=mybir.AxisListType.X,
                        op=mybir.AluOpType.add,
                    )
                    rec = ep_pool.tile([P, 1], f32, tag="rec")
                    nc.vector.reciprocal(rec[:], ssum[:])
                    wout = ep_pool.tile([P, TOPK], f32, tag="wout")
                    nc.vector.tensor_scalar(
                        wout[:],
                        sel[:],
                        rec[:],
                        ROUTE_SCALE,
                        op0=mybir.AluOpType.mult,
                        op1=mybir.AluOpType.mult,
                    )
                    nc.scalar.dma_start(ow_d[st], wout[:])
                    nc.scalar.dma_start(oi_d[st], idx[:])

    nc.compile()
    return nc


def _tile_x(x_shard):
    # [2048, D] -> [16, 128(tok), 56(d_out), 128(d_in)] -> [16, 128(d_in), 56, 128(tok)]
    return x_shard.reshape(NTILES, P, KC, P).transpose(0, 3, 2, 1)


_IDENT = np.eye(P, dtype=np.float32)


def _prep_core(x_shard, wh_t, bb):
    """P1 inputs + host-side tiled arrays kept for the P2 gather."""
    import ml_dtypes

    f8 = ml_dtypes.float8_e4m3
    xs = (x_shard * X_SCALE).astype(np.float32)
    xh = xs.astype(np.float16)
    xl = xs - xh.astype(np.float32)
    xh_t = np.ascontiguousarray(_tile_x(xh))
    xl8_t = np.ascontiguousarray(_tile_x((xl * S_XL).astype(f8)))
    p1_in = {"xh": xh_t, "wh": wh_t, "bb": bb, "ident": _IDENT}
    return p1_in, xh_t, xl8_t


def _k_major(a):
    # [P, (2,) KC, N] -> [KC, P, (2,) N]
    if a.ndim == 3:
        return np.ascontiguousarray(a.transpose(1, 0, 2))
    return np.ascontiguousarray(a.transpose(2, 0, 1, 3))


def _prep_all(x, w, bias):
    import ml_dtypes

    f8 = ml_dtypes.float8_e4m3

    def _tile_w(warr):
        # [256, 7168] -> [128(d_in), 56(d_out), 256(exp)]
        return np.ascontiguousarray(warr.reshape(NEXP, KC, P).transpose(2, 1, 0))

    ws = (w * W_SCALE).astype(np.float32)
    wh = ws.astype(np.float16)
    wl = ws - wh.astype(np.float32)
    wh_t = _k_major(_tile_w(wh))
    wl8 = _tile_w((wl * S_WL).astype(f8))             # pairs fp8(xh)
    wh8 = _tile_w((ws * S_WH).astype(f8))             # pairs xl8
    w8 = _k_major(np.stack([wl8, wh8], axis=1))
    bb = np.ascontiguousarray(np.broadcast_to(bias, (P, NEXP)).astype(np.float32))
    # bias_col[p, h] = bias[h*128 + p]
    bias_col = np.ascontiguousarray(bias.reshape(2, P).T.astype(np.float32))

    with ThreadPoolExecutor(NCORES) as pool:
        cores = list(
            pool.map(
                lambda c: _prep_core(x[c * TPC : (c + 1) * TPC], wh_t, bb),
                range(NCORES),
            )
        )
    return cores, w8, bias_col


def _gather_p2_inputs(core_prep, p1_out, w8, bias_col):
    """Host gather of the selected tokens' fp8 data + raw scores."""
    import ml_dtypes

    f8 = ml_dtypes.float8_e4m3
    _, xh_t, xl8_t = core_prep
    m = np.asarray(p1_out["out_map"], np.int64)        # [16, 16] token-in-tile
    tiles = np.repeat(np.arange(NTILES), CAP)          # [256]
    toks = m.reshape(-1)                               # [256]
    # [256, 128, 56] -> [128, 56, 256]
    xh_g = xh_t[tiles, :, :, toks].transpose(1, 2, 0)
    xl8_g = xl8_t[tiles, :, :, toks].transpose(1, 2, 0)
    x8g = np.empty((P, 2, KC, NSLOT), f8)
    x8g[:, 0] = xh_g.astype(f8)
    x8g[:, 1] = xl8_g
    x8g = _k_major(x8g)
    sraw = np.asarray(p1_out["sraw"])                  # [16, 128, 256]
    sg = sraw[tiles, toks]                             # [256 slots, 256 exp]
    sgT = np.ascontiguousarray(
        sg.T.reshape(2, P, NSLOT).transpose(1, 0, 2)
    )  # [128, 2, 256]
    return {
        "w8": w8,
        "x8g": x8g,
        "sgT": sgT,
        "bias_col": bias_col,
        "ident": _IDENT,
    }, tiles, toks


def _merge(p1_results, p2_results, maps):
    weights = np.concatenate(
        [np.asarray(r["out_w"]).reshape(TPC, TOPK) for r in p1_results], axis=0
    ).astype(np.float32)
    indices = np.concatenate(
        [np.asarray(r["out_i"]).reshape(TPC, TOPK) for r in p1_results], axis=0
    ).astype(np.int32)
    for c, (r2, (tiles, toks)) in enumerate(zip(p2_results, maps)):
        rows = c * TPC + tiles * P + toks
        weights[rows] = np.asarray(r2["ow2"]).reshape(NSLOT, TOPK)
        indices[rows] = np.asarray(r2["oi2"]).reshape(NSLOT, TOPK).astype(np.int32)
    return weights, indices


def kernel(**inputs):
    from concourse.bass_utils import run_bass_kernel_spmd

    x = np.ascontiguousarray(np.asarray(inputs["x"], dtype=np.float32))
    w = np.ascontiguousarray(np.asarray(inputs["weight"], dtype=np.float32))
    bias = np.asarray(inputs["bias"], dtype=np.float32)

    cores, w8, bias_col = _prep_all(x, w, bias)

    nc1 = _build_p1()
    r1 = run_bass_kernel_spmd(
        nc1, [c[0] for c in cores], core_ids=list(range(NCORES)), trace=False
    ).results

    p2_maps = []
    p2_ins = []
    for c in range(NCORES):
        p2_in, tiles, toks = _gather_p2_inputs(cores[c], r1[c], w8, bias_col)
        p2_ins.append(p2_in)
        p2_maps.append((tiles, toks))

    nc2 = _build_p2()
    r2 = run_bass_kernel_spmd(
        nc2, p2_ins, core_ids=list(range(NCORES)), trace=False
    ).results

    return _merge(r1, r2, p2_maps)
